# revision 3
# baseline (speedup 1.0000x reference)
"""Segment mean-pool (LocalPooling1D) Trainium2 Bass kernel.

x [32, 8192, 256] f32, x_pos [32, 65] sorted int32 boundaries -> y [32, 64, 256].
y[b, j] = mean(x[b, x_pos[b,j]:x_pos[b,j+1]]), empty segments -> 0.

Strategy: data-parallel over batch, 4 rows per core on 8 cores; the kernel is
HBM-bandwidth-bound, so everything is built around a clean x stream.

SDMA engine 15 (the one serving SBUF partitions 92-95/124-127 per the port
swizzle port = bits[4:2]<<1 | bit[6]) runs ~20% slower than engines 0-14 when
the NTFF profiler's drain traffic is active, and every engine's share of a
DMA is fixed by the partition->port map - so a 128-partition stream is gated
by engine 15 finishing its 1/16 of the bytes ~18us late. This kernel spreads
each row's tokens over partitions 0-123 only: engine 15 serves just 4 of its
8 partitions (92-95), i.e. a half share, so even 40% slower it never gates;
engines 0-14 carry 8/124 of the bytes each (+3.2% vs /128). Token t of a row
maps to partition p = t // 66, q = t % 66, so every partition's x data is one
contiguous 66 KB HBM chunk -> uniform DMA descriptors. The 8 leftover tokens
(8184..8191) go one-per-partition on partitions 0-7 as a single extra
accumulation step per row, issued first so it is never on the critical tail.

The 0/1 segment-indicator ind[p, q, j] = (pos[j] <= 66p + q < pos[j+1]) is
built on the DVE per x-chunk, from a [124, 66] q-iota and a [124, P]
broadcast of pos done on the (idle at startup) TensorEngine as
ones[1,124].T @ pos[1,P] - avoiding the gpsimd PartitionBroadcast custom op,
whose first use stalls ~10us on a Q7 library reload. Segment sums accumulate
on the PE as psum += ind_q.T @ x_q in float32r (1 cycle/row, 4x faster than
fp32; ind is exactly 0/1 so only x's low mantissa bits are lost - rel err
~1e-4, tolerance 2e-2).

No SWDGE (gpsimd) DMAs anywhere: SWDGE descriptor-ring traffic contends with
SDMA engines 7/15. The last x chunk of the last row is split into small
pieces so the post-stream matmul burst is short, and outputs are stored with
HWDGE DMAs at the very end.
"""

import os
import sys

import numpy as np

sys.path.insert(0, "/opt/trn_rl_repo")

import concourse.bacc as bacc
import concourse.bass as bass
import concourse.tile as tile
from concourse import mybir
from concourse.bass_utils import run_bass_kernel_spmd

dt = mybir.dt
Alu = mybir.AluOpType

# Problem constants (hardcoded per harness contract).
B, T, C, P = 32, 8192, 256, 65
NSEG = P - 1
NCORES = 8
R = B // NCORES          # batch rows per core

# Engine-15-light layout: tokens on partitions 0-123 only.
PDIM = 124               # partitions used (124-127 idle -> engine 15 half share)
QTOK = 66                # tokens per partition (66 * 124 = 8184)
EXTRA = 8                # leftover tokens 8184..8191 on partitions 0..7

CFG = {
    "chunkq": int(os.environ.get("KB_CHUNKQ", "22")),      # q-slices per x DMA
    "x_bufs": int(os.environ.get("KB_XBUFS", "5")),
    "ind_bufs": int(os.environ.get("KB_INDBUFS", "4")),
    "s_bufs": int(os.environ.get("KB_SBUFS", "3")),
    "psum_bufs": int(os.environ.get("KB_PSUMBUFS", "2")),
    "dual_dma": os.environ.get("KB_DUALDMA", "1") == "1",
}


def build_program(cfg=CFG):
    chunkq = cfg["chunkq"]
    nchunk = QTOK // chunkq
    assert chunkq * nchunk == QTOK

    nc = bacc.Bacc("TRN2", target_bir_lowering=False, debug=False)

    # float32r: same bit layout as f32; enables the 1-cycle/row PE matmul mode
    # (vs 4 for fp32). The BIR verifier requires matmul operand producers to
    # declare f32r output, so x is f32r end-to-end (DMA is then a plain copy).
    x_d = nc.dram_tensor("x", [R, T, C], dt.float32r, kind="ExternalInput")
    pos_d = nc.dram_tensor("x_pos", [R, P], dt.int32, kind="ExternalInput")
    y_d = nc.dram_tensor("y", [R, NSEG, C], dt.float32, kind="ExternalOutput")

    with tile.TileContext(nc) as tc:
        with (
            tc.tile_pool(name="const", bufs=1) as constp,
            tc.tile_pool(name="xp", bufs=cfg["x_bufs"]) as xp,
            tc.tile_pool(name="sp", bufs=cfg["s_bufs"]) as sp,
            tc.tile_pool(name="indp", bufs=cfg["ind_bufs"]) as indp,
            tc.tile_pool(name="smallp", bufs=R) as smallp,
            tc.tile_pool(name="outp", bufs=2) as outp,
            tc.tile_pool(name="psp", bufs=cfg["psum_bufs"], space="PSUM") as psp,
            tc.tile_pool(name="pspos", bufs=1, space="PSUM") as pspos,
            tc.tile_pool(name="xtailp", bufs=2) as xtailp,
            tc.tile_pool(name="stailp", bufs=2) as stailp,
            tc.tile_pool(name="indtailp", bufs=2) as indtailp,
            tc.tile_pool(name="xep", bufs=2) as xep,
        ):
            # q (token index within partition) along the free axis: [PDIM, QTOK].
            q_sm = constp.tile([PDIM, QTOK], dt.float32)
            nc.gpsimd.iota(q_sm[:], pattern=[[1, QTOK]], base=0,
                           channel_multiplier=0, allow_small_or_imprecise_dtypes=True)
            # 66*p as a per-partition scalar (<= 8118, exact in f32).
            tok_base = constp.tile([PDIM, 1], dt.float32)
            nc.gpsimd.iota(tok_base[:], pattern=[[1, 1]], base=0, channel_multiplier=QTOK,
                           allow_small_or_imprecise_dtypes=True)
            # Extra-token base: 8184 + p on partitions 0..EXTRA.
            tok_x = constp.tile([EXTRA, 1], dt.float32)
            nc.gpsimd.iota(tok_x[:], pattern=[[1, 1]], base=PDIM * QTOK,
                           channel_multiplier=1, allow_small_or_imprecise_dtypes=True)

            ones_row = constp.tile([1, PDIM], dt.float32)
            nc.gpsimd.iota(ones_row[:], pattern=[[0, PDIM]], base=1,
                           channel_multiplier=0, allow_small_or_imprecise_dtypes=True)
            ones1 = constp.tile([1, 1], dt.float32, tag="ones1")
            nc.gpsimd.iota(ones1[:], pattern=[[0, 1]], base=1,
                           channel_multiplier=0, allow_small_or_imprecise_dtypes=True)

            # All pos rows in ONE single-descriptor 1 KB DMA on the scalar
            # queue.
            pos_all = smallp.tile([1, R * P], dt.int32, tag="posall")
            nc.scalar.dma_start(
                pos_all[:].rearrange("one (r p) -> one r p", r=R), pos_d[:, :])
            pos_rows = [pos_all[:, r * P : (r + 1) * P] for r in range(R)]

            # Per row: pos broadcast to PDIM partitions on the PE
            # (ones[1,PDIM].T @ pos[1,P]), and segment counts computed in the
            # free axis then transposed to [NSEG, 1] with a K=1 matmul.
            pos_bs, recips = [], []
            for r in range(R):
                posf_row = smallp.tile([1, P], dt.float32, tag="posf")
                nc.vector.tensor_copy(posf_row[:], pos_rows[r])
                ps_pos = pspos.tile([PDIM, P], dt.float32)
                nc.tensor.matmul(ps_pos[:], ones_row[:], posf_row[:],
                                 start=True, stop=True)
                pos_b = smallp.tile([PDIM, P], dt.float32, tag="posb")
                nc.vector.tensor_copy(pos_b[:], ps_pos[:])
                pos_bs.append(pos_b)

                cnt_row = smallp.tile([1, NSEG], dt.float32, tag="cntrow")
                nc.vector.tensor_tensor(
                    cnt_row[:], posf_row[:, 1:P], posf_row[:, 0:NSEG], op=Alu.subtract)
                ps_cnt = pspos.tile([NSEG, 1], dt.float32, tag="cntT")
                nc.tensor.matmul(ps_cnt[:], cnt_row[:], ones1[:],
                                 start=True, stop=True)
                cntc = smallp.tile([NSEG, 1], dt.float32, tag="cntc")
                nc.vector.tensor_scalar(cntc[:], ps_cnt[:], 1.0, None, op0=Alu.max)
                recip = smallp.tile([NSEG, 1], dt.float32, tag="recip")
                nc.vector.reciprocal(recip[:], cntc[:])
                recips.append(recip)

            # All four rows' outputs accumulate here; HWDGE stores at the end.
            y_all = outp.tile([NSEG, R * C], dt.float32)

            for r in range(R):
                pos_b = pos_bs[r]
                ps = psp.tile([NSEG, C], dt.float32)
                # Row as [124 partitions, 66*256]: partition p's line is the
                # contiguous HBM range of tokens [66p, 66p+66).
                xv = x_d[r][0 : PDIM * QTOK, :].rearrange(
                    "(p q) c -> p (q c)", p=PDIM)

                # Extra step first: tokens 8184+p on partitions 0..7, ind from
                # a single-token compare; start=True zeroes the PSUM bank,
                # chunk matmuls then accumulate on top.
                xe = xep.tile([EXTRA, C], dt.float32r, tag="xe")
                nc.sync.dma_start(xe[:], x_d[r][PDIM * QTOK : T, :])
                S_x = smallp.tile([EXTRA, P], dt.float32, tag="sx")
                nc.vector.scalar_tensor_tensor(
                    S_x[:],
                    pos_b[0:EXTRA, :],
                    tok_x[:],
                    q_sm[0:EXTRA, 0:1].broadcast_to((EXTRA, P)),
                    op0=Alu.subtract,
                    op1=Alu.is_le,
                )
                ind_x = smallp.tile([EXTRA, NSEG], dt.float32r, tag="indx")
                nc.vector.tensor_tensor(
                    ind_x[:], S_x[:, 0:NSEG], S_x[:, 1:P], op=Alu.subtract)
                nc.tensor.matmul(ps[:], ind_x[:], xe[:], start=True, stop=False)

                # The very last chunk gates the kernel tail: split it into
                # small pieces so the post-stream burst is short.
                if r == R - 1:
                    qsteps = [chunkq] * (nchunk - 1) + [chunkq // 2, chunkq - chunkq // 2]
                else:
                    qsteps = [chunkq] * nchunk
                q0 = 0
                for ci, cq in enumerate(qsteps):
                    tail = cq != chunkq
                    # S[p, k, j] = (pos[j] <= 66p + q), q = q0 + k.
                    S_c = (stailp if tail else sp).tile([PDIM, cq, P], dt.float32, tag="sall")
                    nc.vector.scalar_tensor_tensor(
                        S_c[:],
                        pos_b[:, None, :].broadcast_to((PDIM, cq, P)),
                        tok_base[:],
                        q_sm[:, q0 : q0 + cq, None].broadcast_to((PDIM, cq, P)),
                        op0=Alu.subtract,
                        op1=Alu.is_le,
                    )
                    # ind[p, k, j] = S[p, k, j] - S[p, k, j+1]
                    ind_c = (indtailp if tail else indp).tile([PDIM, cq, NSEG], dt.float32r, tag="ind")
                    nc.vector.tensor_tensor(
                        ind_c[:], S_c[:, :, 0:NSEG], S_c[:, :, 1:P], op=Alu.subtract
                    )

                    xt = (xtailp if tail else xp).tile([PDIM, cq * C], dt.float32r, tag="x")
                    eng = nc.scalar if (cfg["dual_dma"] and ci % 2) else nc.sync
                    eng.dma_start(xt[:], xv[:, q0 * C : (q0 + cq) * C])
                    for k in range(cq):
                        q = q0 + k
                        rhs = xt[:, k * C : (k + 1) * C]
                        lhsT = ind_c[:, k, :]
                        nc.tensor.matmul(
                            ps[:], lhsT, rhs,
                            start=False, stop=(q == QTOK - 1),
                        )
                    q0 += cq

                out_t = y_all[:, r * C : (r + 1) * C]
                nc.vector.tensor_scalar(out_t, ps[:], recips[r][:], None, op0=Alu.mult)

            # Per-row HWDGE stores, all issued after the last x issue so they
            # block nothing; rows 0-2 complete while the last row still
            # computes, leaving only row 3's 64 KB (+receipt) in the tail.
            for r in range(R):
                eng = nc.scalar if r % 2 else nc.sync
                eng.dma_start(y_d[r], y_all[:, r * C : (r + 1) * C])

    nc.compile()
    return nc


_PROGRAM = None


def _get_program():
    global _PROGRAM
    if _PROGRAM is None:
        _PROGRAM = build_program()
    return _PROGRAM


def kernel(x, x_pos):
    x = np.ascontiguousarray(x, dtype=np.float32)
    x_pos = np.ascontiguousarray(x_pos, dtype=np.int32)
    nc = _get_program()
    in_maps = [
        {"x": x[c * R : (c + 1) * R], "x_pos": x_pos[c * R : (c + 1) * R]}
        for c in range(NCORES)
    ]
    res = run_bass_kernel_spmd(nc, in_maps, list(range(NCORES)))
    y = np.concatenate([res.results[c]["y"] for c in range(NCORES)], axis=0)
    return y.astype(np.float32)


# revision 5
# speedup vs baseline: 3.1766x; 3.1766x over previous
"""Segment mean-pool (LocalPooling1D) Trainium2 Bass kernel.

x [32, 8192, 256] f32, x_pos [32, 65] sorted int32 boundaries -> y [32, 64, 256].
y[b, j] = mean(x[b, x_pos[b,j]:x_pos[b,j+1]]), empty segments -> 0.

Strategy: data-parallel over batch, 4 rows per core on 8 cores; the kernel is
HBM-bandwidth-bound, so everything is built around a clean ~432 GB/s x stream.

Token t of a row maps to SBUF partition p = t // 64, free-slot q = t % 64, so
every partition's x data is one contiguous 64 KB HBM chunk -> uniform 16 KB
DMA descriptors. The HWDGE deals a DMA's descriptors over SDMA engines in
G = (largest power of two <= 16 dividing the outer count) contiguous groups
onto engines 0..G-1: a 128-partition DMA engages all 16 engines, 8 partitions
each. SDMA engine 15 runs ~20% slower than engines 0-14 while the NTFF
profiler's drain traffic is active (uniform-in-time per-descriptor stretch,
measured 700-840ns vs 606ns for a 16KB descriptor), so with a pure 16-way
stream the kernel end waits ~18us for engine 15's share. Countermeasure: the
last ~14% of the stream (the tail of batch row 3) is issued as PAIRS of
8-way DMAs (partition slices [0:56) and [56:128), both outer counts divisible
by 8 but not 16 -> engines 0-7 only). Engine 15 then finishes its reduced
16-way share before engines 0-7 finish their extra tail work, and the
straggler never gates: stream ~= (1+f)*X/16/27GB/s with f=0.14, vs X/16/(27*
0.78 GB/s) when engine 15 gates.

The 0/1 segment-indicator ind[p, q, j] = (pos[j] <= 64p + q < pos[j+1]) is
built on the DVE per x-chunk, from a tiny [128, 64] q-iota and a [128, P]
broadcast of pos done on the (idle at startup) TensorEngine as
ones[1,128].T @ pos[1,P] - avoiding the gpsimd PartitionBroadcast custom op,
whose first use stalls ~10us on a Q7 library reload. Segment sums accumulate
on the PE as psum += ind_q.T @ x_q in float32r (1 cycle/row at N=256, 4x
faster than fp32; ind is exactly 0/1 so only x's low mantissa bits are lost -
rel err ~1e-4, tolerance 2e-2).

No SWDGE (gpsimd) DMAs anywhere. Rows 0-2's outputs are stored as soon as
each row's scale completes (their 16-way descriptors and HBM receipts drain
inside the stream); only row 3's 64 KB store (+receipt) is in the tail.
"""

import os
import sys

import numpy as np

sys.path.insert(0, "/opt/trn_rl_repo")

import concourse.bacc as bacc
import concourse.bass as bass
import concourse.tile as tile
from concourse import mybir
from concourse.bass_utils import run_bass_kernel_spmd

dt = mybir.dt
Alu = mybir.AluOpType

# Problem constants (hardcoded per harness contract).
B, T, C, P = 32, 8192, 256, 65
NSEG = P - 1
NCORES = 8
R = B // NCORES          # batch rows per core
NPART = 128              # SBUF partitions
QTOK = T // NPART        # 64 tokens per partition (contiguous in HBM)
W8SPLIT = 56             # 8-way chunk partition split: [0:56) + [56:128)

CFG = {
    "chunkq": int(os.environ.get("KB_CHUNKQ", "16")),      # q-slices per x DMA
    "x_bufs": int(os.environ.get("KB_XBUFS", "5")),
    "ind_bufs": int(os.environ.get("KB_INDBUFS", "5")),
    "s_bufs": int(os.environ.get("KB_SBUFS", "3")),
    "psum_bufs": int(os.environ.get("KB_PSUMBUFS", "2")),
    "dual_dma": os.environ.get("KB_DUALDMA", "1") == "1",
    # row-3 chunk schedule: (q-count, use 8-way split) — 16+12+8+8+8+6+6 = 64
    "tail_sched": ((16, False), (12, False), (8, True), (8, True), (8, True),
                   (6, True), (6, True)),
}


def build_program(cfg=CFG):
    chunkq = cfg["chunkq"]
    nchunk = QTOK // chunkq

    nc = bacc.Bacc("TRN2", target_bir_lowering=False, debug=False)

    # float32r: same bit layout as f32; enables the 1-cycle/row PE matmul mode
    # (vs 4 for fp32). The BIR verifier requires matmul operand producers to
    # declare f32r output, so x is f32r end-to-end (DMA is then a plain copy).
    x_d = nc.dram_tensor("x", [R, T, C], dt.float32r, kind="ExternalInput")
    pos_d = nc.dram_tensor("x_pos", [R, P], dt.int32, kind="ExternalInput")
    y_d = nc.dram_tensor("y", [R, NSEG, C], dt.float32, kind="ExternalOutput")

    with tile.TileContext(nc) as tc:
        with (
            tc.tile_pool(name="const", bufs=1) as constp,
            tc.tile_pool(name="xp", bufs=cfg["x_bufs"]) as xp,
            tc.tile_pool(name="sp", bufs=cfg["s_bufs"]) as sp,
            tc.tile_pool(name="indp", bufs=cfg["ind_bufs"]) as indp,
            tc.tile_pool(name="smallp", bufs=R) as smallp,
            tc.tile_pool(name="outp", bufs=2) as outp,
            tc.tile_pool(name="psp", bufs=cfg["psum_bufs"], space="PSUM") as psp,
            tc.tile_pool(name="pspos", bufs=1, space="PSUM") as pspos,
            tc.tile_pool(name="xtailp", bufs=3) as xtailp,
            tc.tile_pool(name="stailp", bufs=3) as stailp,
            tc.tile_pool(name="indtailp", bufs=3) as indtailp,
        ):
            # q (token index within partition) along the free axis: [128, 64].
            q_sm = constp.tile([NPART, QTOK], dt.float32)
            nc.gpsimd.iota(q_sm[:], pattern=[[1, QTOK]], base=0,
                           channel_multiplier=0, allow_small_or_imprecise_dtypes=True)
            # 64*p as a per-partition scalar (<= 8128, exact in f32).
            p64_iota = constp.tile([NPART, 1], dt.float32)
            nc.gpsimd.iota(p64_iota[:], pattern=[[1, 1]], base=0, channel_multiplier=QTOK,
                           allow_small_or_imprecise_dtypes=True)
            ones_row = constp.tile([1, NPART], dt.float32)
            nc.gpsimd.iota(ones_row[:], pattern=[[0, NPART]], base=1,
                           channel_multiplier=0, allow_small_or_imprecise_dtypes=True)

            ones1 = constp.tile([1, 1], dt.float32, tag="ones1")
            nc.gpsimd.iota(ones1[:], pattern=[[0, 1]], base=1,
                           channel_multiplier=0, allow_small_or_imprecise_dtypes=True)

            # All pos rows in ONE single-descriptor 1 KB DMA on the scalar
            # queue. The sync queue starts directly with x chunk 0.
            pos_all = smallp.tile([1, R * P], dt.int32, tag="posall")
            nc.scalar.dma_start(
                pos_all[:].rearrange("one (r p) -> one r p", r=R), pos_d[:, :])
            pos_rows = [pos_all[:, r * P : (r + 1) * P] for r in range(R)]

            # Per row: pos broadcast to 128 partitions on the PE
            # (ones[1,128].T @ pos[1,P]), and segment counts computed in the
            # free axis then transposed to [NSEG, 1] with a K=1 matmul.
            pos_bs, recips = [], []
            for r in range(R):
                posf_row = smallp.tile([1, P], dt.float32, tag="posf")
                nc.vector.tensor_copy(posf_row[:], pos_rows[r])
                ps_pos = pspos.tile([NPART, P], dt.float32)
                nc.tensor.matmul(ps_pos[:], ones_row[:], posf_row[:],
                                 start=True, stop=True)
                pos_b = smallp.tile([NPART, P], dt.float32, tag="posb")
                nc.vector.tensor_copy(pos_b[:], ps_pos[:])
                pos_bs.append(pos_b)

                cnt_row = smallp.tile([1, NSEG], dt.float32, tag="cntrow")
                nc.vector.tensor_tensor(
                    cnt_row[:], posf_row[:, 1:P], posf_row[:, 0:NSEG], op=Alu.subtract)
                ps_cnt = pspos.tile([NSEG, 1], dt.float32, tag="cntT")
                nc.tensor.matmul(ps_cnt[:], cnt_row[:], ones1[:],
                                 start=True, stop=True)
                cntc = smallp.tile([NSEG, 1], dt.float32, tag="cntc")
                nc.vector.tensor_scalar(cntc[:], ps_cnt[:], 1.0, None, op0=Alu.max)
                recip = smallp.tile([NSEG, 1], dt.float32, tag="recip")
                nc.vector.reciprocal(recip[:], cntc[:])
                recips.append(recip)

            # All four rows' outputs accumulate here.
            y_all = outp.tile([NSEG, R * C], dt.float32)

            for r in range(R):
                pos_b = pos_bs[r]
                ps = psp.tile([NSEG, C], dt.float32)
                # Row as [128 partitions, 64*256]: partition p's line is the
                # contiguous HBM range of tokens [64p, 64p+64).
                xr = x_d[r].rearrange("(p q) c -> p (q c)", p=NPART)
                if r == R - 1:
                    qsteps = list(cfg["tail_sched"])
                else:
                    qsteps = [(chunkq, False)] * nchunk
                q0 = 0
                for ci, (cq, w8) in enumerate(qsteps):
                    tail = cq != chunkq
                    # S[p, k, j] = (pos[j] <= 64p + q), q = q0 + k.
                    S_c = (stailp if tail else sp).tile([NPART, cq, P], dt.float32, tag="sall")
                    nc.vector.scalar_tensor_tensor(
                        S_c[:],
                        pos_b[:, None, :].broadcast_to((NPART, cq, P)),
                        p64_iota[:],
                        q_sm[:, q0 : q0 + cq, None].broadcast_to((NPART, cq, P)),
                        op0=Alu.subtract,
                        op1=Alu.is_le,
                    )
                    # ind[p, k, j] = S[p, k, j] - S[p, k, j+1]
                    ind_c = (indtailp if tail else indp).tile([NPART, cq, NSEG], dt.float32r, tag="ind")
                    nc.vector.tensor_tensor(
                        ind_c[:], S_c[:, :, 0:NSEG], S_c[:, :, 1:P], op=Alu.subtract
                    )

                    xt = (xtailp if tail else xp).tile([NPART, cq * C], dt.float32r, tag="x")
                    eng = nc.scalar if (cfg["dual_dma"] and ci % 2) else nc.sync
                    src = xr[:, q0 * C : (q0 + cq) * C]
                    if w8:
                        # Two 8-way deals (56 and 72 partitions): engines 0-7
                        # only, keeping the straggling engine 15 off the tail.
                        eng.dma_start(xt[0:W8SPLIT, :], src[0:W8SPLIT, :])
                        eng.dma_start(xt[W8SPLIT:NPART, :], src[W8SPLIT:NPART, :])
                    else:
                        eng.dma_start(xt[:], src)
                    for k in range(cq):
                        q = q0 + k
                        rhs = xt[:, k * C : (k + 1) * C]
                        lhsT = ind_c[:, k, :]
                        nc.tensor.matmul(
                            ps[:], lhsT, rhs,
                            start=(q == 0), stop=(q == QTOK - 1),
                        )
                    q0 += cq

                out_t = y_all[:, r * C : (r + 1) * C]
                nc.vector.tensor_scalar(out_t, ps[:], recips[r][:], None, op0=Alu.mult)
                # Store this row now: rows 0-2's 16-way store descriptors and
                # HBM receipts drain inside the stream; only row 3's store is
                # in the tail.
                eng = nc.scalar if r % 2 else nc.sync
                eng.dma_start(y_d[r], out_t)

    nc.compile()
    return nc


_PROGRAM = None


def _get_program():
    global _PROGRAM
    if _PROGRAM is None:
        _PROGRAM = build_program()
    return _PROGRAM


def kernel(x, x_pos):
    x = np.ascontiguousarray(x, dtype=np.float32)
    x_pos = np.ascontiguousarray(x_pos, dtype=np.int32)
    nc = _get_program()
    in_maps = [
        {"x": x[c * R : (c + 1) * R], "x_pos": x_pos[c * R : (c + 1) * R]}
        for c in range(NCORES)
    ]
    res = run_bass_kernel_spmd(nc, in_maps, list(range(NCORES)))
    y = np.concatenate([res.results[c]["y"] for c in range(NCORES)], axis=0)
    return y.astype(np.float32)


# revision 6
# speedup vs baseline: 3.2548x; 1.0246x over previous
"""Segment mean-pool (LocalPooling1D) Trainium2 Bass kernel.

x [32, 8192, 256] f32, x_pos [32, 65] sorted int32 boundaries -> y [32, 64, 256].
y[b, j] = mean(x[b, x_pos[b,j]:x_pos[b,j+1]]), empty segments -> 0.

Strategy: data-parallel over batch, 4 rows per core on 8 cores; the kernel is
HBM-bandwidth-bound, so everything is built around a clean ~432 GB/s x stream.

Token t of a row maps to SBUF partition p = t // 64, free-slot q = t % 64, so
every partition's x data is one contiguous 64 KB HBM chunk -> uniform 16 KB
DMA descriptors. The HWDGE deals a DMA's descriptors over SDMA engines in
G = (largest power of two <= 16 dividing the outer count) contiguous groups
onto engines 0..G-1: a 128-partition DMA engages all 16 engines, 8 partitions
each. SDMA engine 15 runs ~20% slower than engines 0-14 while the NTFF
profiler's drain traffic is active (uniform-in-time per-descriptor stretch,
measured 700-840ns vs 606ns for a 16KB descriptor), so with a pure 16-way
stream the kernel end waits ~18us for engine 15's share. Countermeasure: the
last ~14% of the stream (the tail of batch row 3) is issued as PAIRS of
8-way DMAs (partition slices [0:56) and [56:128), both outer counts divisible
by 8 but not 16 -> engines 0-7 only). Engine 15 then finishes its reduced
16-way share before engines 0-7 finish their extra tail work, and the
straggler never gates: stream ~= (1+f)*X/16/27GB/s with f=0.14, vs X/16/(27*
0.78 GB/s) when engine 15 gates.

The 0/1 segment-indicator ind[p, q, j] = (pos[j] <= 64p + q < pos[j+1]) is
built on the DVE per x-chunk, from a tiny [128, 64] q-iota and a [128, P]
broadcast of pos done on the (idle at startup) TensorEngine as
ones[1,128].T @ pos[1,P] - avoiding the gpsimd PartitionBroadcast custom op,
whose first use stalls ~10us on a Q7 library reload. Segment sums accumulate
on the PE as psum += ind_q.T @ x_q in float32r (1 cycle/row at N=256, 4x
faster than fp32; ind is exactly 0/1 so only x's low mantissa bits are lost -
rel err ~1e-4, tolerance 2e-2).

No SWDGE (gpsimd) DMAs anywhere. Rows 0-2's outputs are stored as soon as
each row's scale completes (their 16-way descriptors and HBM receipts drain
inside the stream); only row 3's 64 KB store (+receipt) is in the tail.
"""

import os
import sys

import numpy as np

sys.path.insert(0, "/opt/trn_rl_repo")

import concourse.bacc as bacc
import concourse.bass as bass
import concourse.tile as tile
from concourse import mybir
from concourse.bass_utils import run_bass_kernel_spmd

dt = mybir.dt
Alu = mybir.AluOpType

# Problem constants (hardcoded per harness contract).
B, T, C, P = 32, 8192, 256, 65
NSEG = P - 1
NCORES = 8
R = B // NCORES          # batch rows per core
NPART = 128              # SBUF partitions
QTOK = T // NPART        # 64 tokens per partition (contiguous in HBM)
W8SPLIT = 56             # 8-way chunk partition split: [0:56) + [56:128)

CFG = {
    "chunkq": int(os.environ.get("KB_CHUNKQ", "16")),      # q-slices per x DMA
    "x_bufs": int(os.environ.get("KB_XBUFS", "7")),
    "ind_bufs": int(os.environ.get("KB_INDBUFS", "5")),
    "s_bufs": int(os.environ.get("KB_SBUFS", "3")),
    "psum_bufs": int(os.environ.get("KB_PSUMBUFS", "2")),
    "dual_dma": os.environ.get("KB_DUALDMA", "1") == "1",
    # last-rows chunk schedules: (q-count, use 8-way split); 8-way chunks
    # carry f = 48/256 = 18.75%% of the stream on engines 0-13 only.
    "sched_r2": ((16, False), (16, False), (16, False), (8, True), (8, True)),
    "sched_r3": ((16, False), (16, False), (8, True), (8, True), (8, True), (8, True)),
}


def build_program(cfg=CFG):
    chunkq = cfg["chunkq"]
    nchunk = QTOK // chunkq

    nc = bacc.Bacc("TRN2", target_bir_lowering=False, debug=False)

    # float32r: same bit layout as f32; enables the 1-cycle/row PE matmul mode
    # (vs 4 for fp32). The BIR verifier requires matmul operand producers to
    # declare f32r output, so x is f32r end-to-end (DMA is then a plain copy).
    x_d = nc.dram_tensor("x", [R, T, C], dt.float32r, kind="ExternalInput")
    pos_d = nc.dram_tensor("x_pos", [R, P], dt.int32, kind="ExternalInput")
    y_d = nc.dram_tensor("y", [R, NSEG, C], dt.float32, kind="ExternalOutput")

    with tile.TileContext(nc) as tc:
        with (
            tc.tile_pool(name="const", bufs=1) as constp,
            tc.tile_pool(name="xp", bufs=cfg["x_bufs"]) as xp,
            tc.tile_pool(name="sp", bufs=cfg["s_bufs"]) as sp,
            tc.tile_pool(name="indp", bufs=cfg["ind_bufs"]) as indp,
            tc.tile_pool(name="smallp", bufs=R) as smallp,
            tc.tile_pool(name="outp", bufs=2) as outp,
            tc.tile_pool(name="psp", bufs=cfg["psum_bufs"], space="PSUM") as psp,
            tc.tile_pool(name="pspos", bufs=1, space="PSUM") as pspos,
            tc.tile_pool(name="xtailp", bufs=3) as xtailp,
            tc.tile_pool(name="stailp", bufs=3) as stailp,
            tc.tile_pool(name="indtailp", bufs=3) as indtailp,
        ):
            # q (token index within partition) along the free axis: [128, 64].
            q_sm = constp.tile([NPART, QTOK], dt.float32)
            nc.gpsimd.iota(q_sm[:], pattern=[[1, QTOK]], base=0,
                           channel_multiplier=0, allow_small_or_imprecise_dtypes=True)
            # 64*p as a per-partition scalar (<= 8128, exact in f32).
            p64_iota = constp.tile([NPART, 1], dt.float32)
            nc.gpsimd.iota(p64_iota[:], pattern=[[1, 1]], base=0, channel_multiplier=QTOK,
                           allow_small_or_imprecise_dtypes=True)
            ones_row = constp.tile([1, NPART], dt.float32)
            nc.gpsimd.iota(ones_row[:], pattern=[[0, NPART]], base=1,
                           channel_multiplier=0, allow_small_or_imprecise_dtypes=True)

            ones1 = constp.tile([1, 1], dt.float32, tag="ones1")
            nc.gpsimd.iota(ones1[:], pattern=[[0, 1]], base=1,
                           channel_multiplier=0, allow_small_or_imprecise_dtypes=True)

            # All pos rows in ONE single-descriptor 1 KB DMA on the scalar
            # queue. The sync queue starts directly with x chunk 0.
            pos_all = smallp.tile([1, R * P], dt.int32, tag="posall")
            nc.scalar.dma_start(
                pos_all[:].rearrange("one (r p) -> one r p", r=R), pos_d[:, :])
            pos_rows = [pos_all[:, r * P : (r + 1) * P] for r in range(R)]

            # Per row: pos broadcast to 128 partitions on the PE
            # (ones[1,128].T @ pos[1,P]), and segment counts computed in the
            # free axis then transposed to [NSEG, 1] with a K=1 matmul.
            pos_bs, recips = [], []
            for r in range(R):
                posf_row = smallp.tile([1, P], dt.float32, tag="posf")
                nc.vector.tensor_copy(posf_row[:], pos_rows[r])
                ps_pos = pspos.tile([NPART, P], dt.float32)
                nc.tensor.matmul(ps_pos[:], ones_row[:], posf_row[:],
                                 start=True, stop=True)
                pos_b = smallp.tile([NPART, P], dt.float32, tag="posb")
                nc.vector.tensor_copy(pos_b[:], ps_pos[:])
                pos_bs.append(pos_b)

                cnt_row = smallp.tile([1, NSEG], dt.float32, tag="cntrow")
                nc.vector.tensor_tensor(
                    cnt_row[:], posf_row[:, 1:P], posf_row[:, 0:NSEG], op=Alu.subtract)
                ps_cnt = pspos.tile([NSEG, 1], dt.float32, tag="cntT")
                nc.tensor.matmul(ps_cnt[:], cnt_row[:], ones1[:],
                                 start=True, stop=True)
                cntc = smallp.tile([NSEG, 1], dt.float32, tag="cntc")
                nc.vector.tensor_scalar(cntc[:], ps_cnt[:], 1.0, None, op0=Alu.max)
                recip = smallp.tile([NSEG, 1], dt.float32, tag="recip")
                nc.vector.reciprocal(recip[:], cntc[:])
                recips.append(recip)

            # All four rows' outputs accumulate here.
            y_all = outp.tile([NSEG, R * C], dt.float32)

            for r in range(R):
                pos_b = pos_bs[r]
                ps = psp.tile([NSEG, C], dt.float32)
                # Row as [128 partitions, 64*256]: partition p's line is the
                # contiguous HBM range of tokens [64p, 64p+64).
                xr = x_d[r].rearrange("(p q) c -> p (q c)", p=NPART)
                if r == R - 1:
                    qsteps = list(cfg["sched_r3"])
                elif r == R - 2:
                    qsteps = list(cfg["sched_r2"])
                else:
                    qsteps = [(chunkq, False)] * nchunk
                q0 = 0
                for ci, (cq, w8) in enumerate(qsteps):
                    tail = cq != chunkq
                    # S[p, k, j] = (pos[j] <= 64p + q), q = q0 + k.
                    S_c = (stailp if tail else sp).tile([NPART, cq, P], dt.float32, tag="sall")
                    nc.vector.scalar_tensor_tensor(
                        S_c[:],
                        pos_b[:, None, :].broadcast_to((NPART, cq, P)),
                        p64_iota[:],
                        q_sm[:, q0 : q0 + cq, None].broadcast_to((NPART, cq, P)),
                        op0=Alu.subtract,
                        op1=Alu.is_le,
                    )
                    # ind[p, k, j] = S[p, k, j] - S[p, k, j+1]
                    ind_c = (indtailp if tail else indp).tile([NPART, cq, NSEG], dt.float32r, tag="ind")
                    nc.vector.tensor_tensor(
                        ind_c[:], S_c[:, :, 0:NSEG], S_c[:, :, 1:P], op=Alu.subtract
                    )

                    xt = (xtailp if tail else xp).tile([NPART, cq * C], dt.float32r, tag="x")
                    eng = nc.scalar if (cfg["dual_dma"] and ci % 2) else nc.sync
                    src = xr[:, q0 * C : (q0 + cq) * C]
                    if w8:
                        # Two 8-way deals (56 and 72 partitions): engines 0-7
                        # only, keeping the straggling engine 15 off the tail.
                        eng.dma_start(xt[0:W8SPLIT, :], src[0:W8SPLIT, :])
                        eng.dma_start(xt[W8SPLIT:NPART, :], src[W8SPLIT:NPART, :])
                    else:
                        eng.dma_start(xt[:], src)
                    for k in range(cq):
                        q = q0 + k
                        rhs = xt[:, k * C : (k + 1) * C]
                        lhsT = ind_c[:, k, :]
                        nc.tensor.matmul(
                            ps[:], lhsT, rhs,
                            start=(q == 0), stop=(q == QTOK - 1),
                        )
                    q0 += cq

                out_t = y_all[:, r * C : (r + 1) * C]
                nc.vector.tensor_scalar(out_t, ps[:], recips[r][:], None, op0=Alu.mult)

            # Per-row HWDGE stores, all issued after the last x issue: a store
            # waits on its row's compute, and ANY compute-dependent
            # instruction placed before an x dma_start in sequencer program
            # order serializes the rest of the stream behind compute
            # (measured: +12us engine idle when stores sat between rows).
            for r in range(R):
                eng = nc.scalar if r % 2 else nc.sync
                eng.dma_start(y_d[r], y_all[:, r * C : (r + 1) * C])

    nc.compile()
    return nc


_PROGRAM = None


def _get_program():
    global _PROGRAM
    if _PROGRAM is None:
        _PROGRAM = build_program()
    return _PROGRAM


def kernel(x, x_pos):
    x = np.ascontiguousarray(x, dtype=np.float32)
    x_pos = np.ascontiguousarray(x_pos, dtype=np.int32)
    nc = _get_program()
    in_maps = [
        {"x": x[c * R : (c + 1) * R], "x_pos": x_pos[c * R : (c + 1) * R]}
        for c in range(NCORES)
    ]
    res = run_bass_kernel_spmd(nc, in_maps, list(range(NCORES)))
    y = np.concatenate([res.results[c]["y"] for c in range(NCORES)], axis=0)
    return y.astype(np.float32)
